# revision 1
# baseline (speedup 1.0000x reference)
"""CrystalEncoder Trainium2 kernel.

Strategy: pure data parallel — one crystal (batch element) per NeuronCore.
All O(N^2) work (pairwise distances, RBF expansion, gated message passing)
runs on-device in a single fused Bass/Tile kernel; the host only does O(N)
input prep (embedding gather, operand packing) and the final (B,H)->(B,LAT)
projections.

Device dataflow per core (N=256 atoms, H=128, BINS=40, NL=2):
  1. D2[i,j] = |c_i|^2 + |c_j|^2 + 1e-6 - 2 c_i.c_j  via one K=5 matmul
     (two 128-row i-tiles), Relu clamp, dist = sqrt(D2), both on ACT.
  2. RBF exponents for all 40 bins at once via a K=4 matmul over rows
     (d^2, d) per group: E[(k,g), p] = -gamma*d_p^2 + 2*gamma*c_k*d_p,
     bias -gamma*c_k^2 folded into the Exp activation; pairs free-major.
     rbfT [128, 32768] bf16 (two 40-bin groups at partition 0/64) resident.
  3. Per layer: gate matmul with edge_w stationary (K=40, bf16);
     softplus as Exp then Ln(x+1) (one shared ACT table set);
     DVE multiply by broadcast h_j; segmented reduce over j -> aggT;
     node update zT = node_w^T @ aggT (K=128 f32 matmul) + Silu + mask.
  4. Pooling: reduce over atoms -> sum_h [H, 1] -> DRAM.
Host: g = sum_h / (n_valid + 1e-6); mu / log_var projections.

Sync discipline: this walrus build supports at most ONE semaphore wait per
instruction. All DMAs are issued on gpsimd (SWDGE, single queue => single
sem proc); "dep nops" (engine nop carrying input APs, the same idiom
tile.py uses for debug callbacks) pre-observe producer ticks so no
instruction ever needs two waits.
"""

import numpy as np
import ml_dtypes

B, N, H, LAT, NL, BINS = 8, 256, 128, 64, 2, 40
VMAX = 8.0
GAMMA = 1.0 / (VMAX / BINS) ** 2  # 25.0

G = 2                 # 40-bin groups at partition offsets 0 / 64
IPG = N // G          # 128 i-rows per group
LOCF = IPG * N        # 32768 pairs per group (free size of rbfT)
NFILL = 4             # rf staging buffer fills per group-range
FILLF = LOCF // NFILL  # 8192 pairs per rf fill (32 i-rows)
ECHUNK = 2048         # pairs per Exp activation in rbf stage
CHUNK = 2048          # pairs per gate chunk (8 i-rows)
NCHUNK = (N * N) // CHUNK
CPG = NCHUNK // G     # chunks per group
IPC = CHUNK // N      # i-rows per chunk

_CACHE = {}


def _install_wait_splitter():
    """This walrus build supports at most ONE semaphore wait per ISA
    instruction. Split every multi-wait instruction by inserting same-engine
    NoOp carriers, each holding one of the waits, immediately before it.
    Semantics are preserved: the engine executes its stream in order, so all
    original wait conditions still hold before the instruction runs."""
    import bass_rust
    import concourse.tile as tile
    from concourse import mybir

    if getattr(tile.TileContext, "_wait_split_installed", False):
        return
    orig = tile.TileContext._lower_ordered_insts
    counter = [0]

    def patched(self, ordered):
        for insts in ordered.values():
            newl = []
            for inst in insts:
                si = inst.sync_info
                ow = list(si.on_wait) if (si is not None and si.on_wait) else []
                if len(ow) > 1 and inst.engine != mybir.EngineType.Unassigned:
                    for w in ow[:-1]:
                        counter[0] += 1
                        nop = bass_rust.InstNoOp(
                            name=f"wsplit_{counter[0]}", ins=[], outs=[]
                        )
                        nop.engine = inst.engine
                        nop.sync_info = bass_rust.SyncInfo(
                            on_wait=[w], on_update=[]
                        )
                        newl.append(nop)
                    inst.sync_info = bass_rust.SyncInfo(
                        on_wait=[ow[-1]], on_update=list(si.on_update or [])
                    )
                newl.append(inst)
            insts[:] = newl
        return orig(self, ordered)

    tile.TileContext._lower_ordered_insts = patched

    def patched_dab(self, tick_clock, wait_clock):
        # Reimplementation of _drain_and_barrier: the kernel-tail drain
        # otherwise carries one wait per proc (11 here). Emit single-wait SP
        # nop carriers covering the global clock, then a bare drain.
        from concourse.vector_clock import ScopedClock

        probe = self.nc.sync.nop()
        wait_clock.add_sem_waits(
            probe.ins, ScopedClock({None: tick_clock.global_clock})
        )
        si = probe.ins.sync_info
        ow = list(si.on_wait) if (si is not None and si.on_wait) else []
        if len(ow) > 1:
            probe.ins.sync_info = bass_rust.SyncInfo(
                on_wait=[ow[0]], on_update=list(si.on_update or [])
            )
            for w in ow[1:]:
                n2 = self.nc.sync.nop()
                n2.ins.sync_info = bass_rust.SyncInfo(on_wait=[w], on_update=[])
        self.nc.sync.drain()
        self.nc.all_engine_barrier()
        popped = self.nc._tile_sem_poison_stack.pop()
        assert popped is self._sem_poison
        self.nc.clear_and_free_semaphores(list(self.sems.allocated().values()))
        self.nc.all_engine_barrier()

    tile.TileContext._drain_and_barrier = patched_dab
    tile.TileContext._wait_split_installed = True


def _build_nc(reps=1):
    import concourse.bass as bass
    import concourse.tile as tile
    from concourse import mybir

    _install_wait_splitter()

    F32 = mybir.dt.float32
    BF16 = mybir.dt.bfloat16
    AF = mybir.ActivationFunctionType
    X = mybir.AxisListType.X
    POOL = mybir.EngineType.Pool

    nc = bass.Bass("TRN2", target_bir_lowering=False, debug=False)

    def dep_nop(engine, aps):
        """Engine-local nop reading `aps`: pulls their producers' ticks into
        the engine's observed clock so later real instructions need at most
        one new semaphore wait."""
        nop = engine.nop(hint="dep").ins
        nop.ins = [engine.lower_ap(ap) for ap in aps]
        return nop

    d_geo = nc.dram_tensor("geo", [5, 2 * N], F32, kind="ExternalInput")
    d_h0T = nc.dram_tensor("h0T", [H, N], F32, kind="ExternalInput")
    d_maskF = nc.dram_tensor("maskF", [H, N], F32, kind="ExternalInput")
    d_cE = nc.dram_tensor("cE", [2 * G, 64 * G], F32, kind="ExternalInput")
    d_cbias = nc.dram_tensor("cbias", [64 * G, 1], F32, kind="ExternalInput")
    d_ewR = nc.dram_tensor("ewR", [64 * G, NL * H], BF16, kind="ExternalInput")
    d_ebT = nc.dram_tensor("ebT", [H, NL], F32, kind="ExternalInput")
    d_nwT = nc.dram_tensor("nwT", [H, NL * H], F32, kind="ExternalInput")
    d_nbT = nc.dram_tensor("nbT", [H, NL], F32, kind="ExternalInput")
    d_sumh = nc.dram_tensor("sumh", [H, 1], F32, kind="ExternalOutput")

    with tile.TileContext(nc) as tc:
        with tc.tile_pool(name="consts", bufs=1) as consts:
            kw = dict(forced_dma_engine=POOL)
            t_geo = consts.tile_from(d_geo[:], **kw)
            t_hT = consts.tile_from(d_h0T[:], **kw)
            t_maskF = consts.tile_from(d_maskF[:], **kw)
            t_cE = consts.tile_from(d_cE[:], **kw)
            t_cbias = consts.tile_from(d_cbias[:], **kw)
            t_ewR = consts.tile_from(d_ewR[:], **kw)
            t_ebT = consts.tile_from(d_ebT[:], **kw)
            t_nwT = consts.tile_from(d_nwT[:], **kw)
            t_nbT = consts.tile_from(d_nbT[:], **kw)

            rbfT = consts.tile([64 * G, LOCF], BF16)

            # every engine pre-observes the (single) DMA proc at its max tick
            dep_nop(nc.tensor, [t_geo[:], t_cE[:], t_ewR[:], t_nwT[:]])
            dep_nop(nc.scalar, [t_cbias[:], t_ebT[:], t_nbT[:]])
            dep_nop(nc.vector, [t_hT[:], t_maskF[:]])

            h00 = consts.tile([H, N], mybir.dt.float32, tag="h00")
            nc.vector.tensor_copy(h00[:], t_hT[:])

            for rep in range(reps):
              if rep > 0:
                # restore initial h (body updates t_hT in place)
                nc.vector.tensor_copy(t_hT[:], h00[:])
              # ---- stage 1+2: distances and resident RBF table ----
              with tc.tile_pool(name="geo", bufs=1) as geo, \
                   tc.tile_pool(name="rfp", bufs=2) as rfp, \
                   tc.tile_pool(name="geop", bufs=2, space="PSUM") as geop:
                  d2c = []
                  dst = []
                  for it in range(2):
                      d2p = geop.tile([128, N], F32, tag="ps")
                      nc.tensor.matmul(
                          d2p[:], t_geo[:, it * 128:(it + 1) * 128],
                          t_geo[:, N:2 * N], start=True, stop=True,
                      )
                      c = geo.tile([128, N], F32, tag=f"d2c{it}")
                      nc.scalar.activation(c[:], d2p[:], AF.Relu)
                      s = geo.tile([128, N], F32, tag=f"dist{it}")
                      nc.scalar.activation(s[:], c[:], AF.Sqrt)
                      d2c.append(c)
                      dst.append(s)

                  ipr = FILLF // N  # i-rows per rf fill
                  for hf in range(NFILL):
                      rf = rfp.tile([2 * G, FILLF], F32, tag="rf")
                      for g in range(G):
                          r0 = hf * ipr
                          nc.gpsimd.dma_start(
                              out=rf[2 * g:2 * g + 1, :],
                              in_=d2c[g][r0:r0 + ipr, :],
                          )
                          nc.gpsimd.dma_start(
                              out=rf[2 * g + 1:2 * g + 2, :],
                              in_=dst[g][r0:r0 + ipr, :],
                          )
                      dep_nop(nc.tensor, [rf[:]])
                      for cc in range(FILLF // ECHUNK):
                          e = geop.tile([64 * G, ECHUNK], F32, tag="ps")
                          for s4 in range(ECHUNK // 512):
                              f0 = cc * ECHUNK + s4 * 512
                              nc.tensor.matmul(
                                  e[:, s4 * 512:(s4 + 1) * 512],
                                  t_cE[:], rf[:, f0:f0 + 512],
                                  start=True, stop=True,
                              )
                          o0 = hf * FILLF + cc * ECHUNK
                          nc.scalar.activation(
                              rbfT[:, o0:o0 + ECHUNK], e[:], AF.Exp,
                              bias=t_cbias[:],
                          )

              # ---- stage 3: message-passing layers ----
              with tc.tile_pool(name="lay", bufs=1) as lay, \
                   tc.tile_pool(name="work", bufs=2) as work, \
                   tc.tile_pool(name="gpp", bufs=2, space="PSUM") as gpp:
                  hmr = lay.tile([H, N], BF16, tag="hmr0")
                  nc.vector.tensor_copy(hmr[:], t_hT[:])
                  for l in range(NL):
                      aggT = lay.tile([H, N], F32, tag=f"agg{l}")
                      for c in range(NCHUNK):
                          g, ci = c // CPG, c % CPG
                          lf = ci * CHUNK
                          gp = gpp.tile([H, CHUNK], F32, tag="gp")
                          for s4 in range(CHUNK // 512):
                              nc.tensor.matmul(
                                  gp[:, s4 * 512:(s4 + 1) * 512],
                                  t_ewR[64 * g:64 * g + BINS, l * H:(l + 1) * H],
                                  rbfT[64 * g:64 * g + BINS,
                                       lf + s4 * 512:lf + (s4 + 1) * 512],
                                  start=True, stop=True,
                              )
                          # softplus(x) = ln(exp(x) + 1); Exp/Ln share a table set
                          gx = work.tile([H, CHUNK], F32, tag="gx")
                          nc.scalar.activation(
                              gx[:], gp[:], AF.Exp, bias=t_ebT[:, l:l + 1],
                          )
                          gt = work.tile([H, CHUNK], BF16, tag="gt")
                          nc.scalar.activation(gt[:], gx[:], AF.Ln, bias=1.0)
                          pp = work.tile([H, CHUNK], BF16, tag="pp")
                          nc.vector.tensor_mul(
                              pp[:].rearrange("p (r c) -> p r c", c=N),
                              gt[:].rearrange("p (r c) -> p r c", c=N),
                              hmr[:, None, :].broadcast_to([H, IPC, N]),
                          )
                          i0 = g * IPG + ci * IPC
                          nc.vector.reduce_sum(
                              out=aggT[:, i0:i0 + IPC],
                              in_=pp[:].rearrange("p (r c) -> p r c", c=N),
                              axis=X,
                          )
                      dep_nop(nc.tensor, [aggT[:]])
                      zp = gpp.tile([H, CHUNK], F32, tag="gp")
                      nc.tensor.matmul(
                          zp[:, :N], t_nwT[:, l * H:(l + 1) * H], aggT[:],
                          start=True, stop=True,
                      )
                      sl = lay.tile([H, N], F32, tag=f"sil{l}")
                      nc.scalar.activation(
                          sl[:], zp[:, :N], AF.Silu, bias=t_nbT[:, l:l + 1],
                      )
                      h2 = lay.tile([H, N], F32, tag=f"h2_{l}")
                      nc.vector.tensor_add(h2[:], t_hT[:], sl[:])
                      nc.vector.tensor_mul(t_hT[:], h2[:], t_maskF[:])
                      if l + 1 < NL:
                          hmr = lay.tile([H, N], BF16, tag=f"hmr{l + 1}")
                          nc.vector.tensor_copy(hmr[:], t_hT[:])

                  sumh = lay.tile([H, 1], F32, tag="sumh")
                  nc.vector.reduce_sum(out=sumh[:], in_=t_hT[:], axis=X)
                  nc.gpsimd.dma_start(out=d_sumh[:], in_=sumh[:])

    return nc


def _get_nc(reps=1):
    key = f"nc{reps}"
    if key not in _CACHE:
        _CACHE[key] = _build_nc(reps)
    return _CACHE[key]


def check_waits(nc, max_waits=1, verbose=True):
    """Report instructions carrying more than `max_waits` semaphore waits."""
    bad = []
    for f in nc.m.functions:
        for bb in f.blocks:
            for ins in bb.instructions:
                si = ins.sync_info
                if si is None:
                    continue
                ow = si.on_wait or []
                if len(ow) > max_waits:
                    bad.append((ins.name, type(ins).__name__, ins.engine,
                                [w.ant_name for w in ow]))
    if verbose:
        for b in bad:
            print("MULTIWAIT:", b)
    return bad


def _shared_inputs(edge_w, edge_b, node_w, node_b):
    centers = np.linspace(0.0, VMAX, BINS).astype(np.float64)
    # groups live at 64-partition-aligned offsets (matmul base-partition rule)
    cE = np.zeros((2 * G, 64 * G), np.float32)
    cbias = np.zeros((64 * G, 1), np.float32)
    ewR = np.zeros((64 * G, NL * H), np.float32)
    for g in range(G):
        cE[2 * g + 0, 64 * g:64 * g + BINS] = -GAMMA
        cE[2 * g + 1, 64 * g:64 * g + BINS] = 2.0 * GAMMA * centers
        cbias[64 * g:64 * g + BINS, 0] = -GAMMA * centers * centers
        for l in range(NL):
            ewR[64 * g:64 * g + BINS, l * H:(l + 1) * H] = edge_w[l]
    ewR = ewR.astype(ml_dtypes.bfloat16)
    ebT = np.ascontiguousarray(edge_b.T).astype(np.float32)      # [H, NL]
    nwT = np.concatenate([node_w[l] for l in range(NL)], axis=1)
    nwT = np.ascontiguousarray(nwT).astype(np.float32)           # [H, NL*H]
    nbT = np.ascontiguousarray(node_b.T).astype(np.float32)      # [H, NL]
    return dict(cE=cE, cbias=cbias, ewR=ewR, ebT=ebT, nwT=nwT, nbT=nbT)


def make_in_maps(atom_types, frac_coords, lattice, mask, emb_table,
                 edge_w, edge_b, node_w, node_b):
    shared = _shared_inputs(edge_w, edge_b, node_w, node_b)
    ones = np.ones(N, np.float32)
    in_maps = []
    for b in range(B):
        cart = (frac_coords[b] @ lattice[b]).astype(np.float32)  # (N, 3)
        nsq = (cart * cart).sum(-1).astype(np.float32)
        # geo[:, :N] = lhsT (-2x, -2y, -2z, 1, |c|^2); geo[:, N:] = rhs
        # (x, y, z, |c|^2 + 1e-6, 1):  D2 = lhsT.T @ rhs
        geo = np.zeros((5, 2 * N), np.float32)
        geo[0, :N] = -2.0 * cart[:, 0]
        geo[1, :N] = -2.0 * cart[:, 1]
        geo[2, :N] = -2.0 * cart[:, 2]
        geo[3, :N] = 1.0
        geo[4, :N] = nsq
        geo[0, N:] = cart[:, 0]
        geo[1, N:] = cart[:, 1]
        geo[2, N:] = cart[:, 2]
        geo[3, N:] = nsq + 1e-6
        geo[4, N:] = 1.0
        types = np.where(mask[b], atom_types[b], 0).astype(np.int64)
        h0T = np.ascontiguousarray(emb_table[types].T).astype(np.float32)
        maskF = np.broadcast_to(
            mask[b].astype(np.float32)[None, :], (H, N)
        ).copy()
        in_maps.append(dict(geo=geo, h0T=h0T, maskF=maskF, **shared))
    return in_maps


def kernel(**inputs):
    from concourse.bass_utils import run_bass_kernel_spmd

    atom_types = np.asarray(inputs["atom_types"])
    frac_coords = np.asarray(inputs["frac_coords"], np.float32)
    lattice = np.asarray(inputs["lattice"], np.float32)
    mask = np.asarray(inputs["mask"]).astype(bool)
    emb_table = np.asarray(inputs["emb_table"], np.float32)
    edge_w = np.asarray(inputs["edge_w"], np.float32)
    edge_b = np.asarray(inputs["edge_b"], np.float32)
    node_w = np.asarray(inputs["node_w"], np.float32)
    node_b = np.asarray(inputs["node_b"], np.float32)
    mu_w = np.asarray(inputs["mu_w"], np.float32)
    mu_b = np.asarray(inputs["mu_b"], np.float32)
    var_w = np.asarray(inputs["var_w"], np.float32)
    var_b = np.asarray(inputs["var_b"], np.float32)

    nc = _get_nc()
    in_maps = make_in_maps(atom_types, frac_coords, lattice, mask, emb_table,
                           edge_w, edge_b, node_w, node_b)
    res = run_bass_kernel_spmd(nc, in_maps, core_ids=list(range(B)))
    sum_h = np.stack([res.results[b]["sumh"][:, 0] for b in range(B)])
    n_valid = mask.sum(1).astype(np.float32)
    g = sum_h / (n_valid[:, None] + 1e-6)
    mu = (g @ mu_w + mu_b).astype(np.float32)
    log_var = (g @ var_w + var_b).astype(np.float32)
    return mu, log_var

